# revision 11
# baseline (speedup 1.0000x reference)
"""Differential attention Trainium2 kernel (Bass/Tile), 8-core SPMD. v2.

reference:
  attn1 = softmax(causal(Q1 K1^T / sqrt(D))) V
  attn2 = softmax(causal(Q2 K2^T / sqrt(D))) V
  out   = attn1 - exp(lambda_log) * attn2
shapes: [B=2, H=12, S=2048, D=128] fp32.

Sharding: B*H = 24 head-batches, 3 per NeuronCore (data/head parallel, no
cross-core comms). Host pre-transposes Q/K to [D, S] layout; device returns
output d-major ([D, S] per head) and the host transposes back.

v2 changes over the v1 baseline (182 us):
 - Scores for a (j, j+1) key-tile pair x both passes go into ONE [128, 2048]
   4-bank PSUM tile, so each ScalarE ACTIVATE covers 2048 columns instead of
   512: the ~352-cycle fixed overhead per ACTIVATE amortizes 4x. ScalarE was
   146 us of ACTIVATE time in v1; the exp stream floor is ~87 us.
 - Full-tile E is written as fp8e4m3 (TRN variant, max 240 -- a free bias of
   -1.5 inside the exp keeps max E ~e^4, and the per-column constant cancels
   exactly in softmax). PV and row-sum matmuls on full tiles then run as
   fp8 DoubleRow pair-matmuls (2 key-tiles contracted per instruction at
   0.5 cyc/row) with V and ones in fp8. QK keeps fp16 (DoubleRow needs a
   256-deep contraction; QK's is D=128).
 - Diagonal tiles (which contain all short causal rows, where fp8 E/V noise
   would not average out) stay fully fp16: packed [128, 1280] score region,
   one exp per pass, per-region fp16 PV/sums.
 - reciprocal_approx_fast (1 DVE op, ~18 significant bits) replaces the
   2-op reciprocal_approx_accurate.

Error budget: fp8 E + fp8 V on long rows -> ~3.6%/sqrt(n_eff) relative,
measured end-to-end ~5e-3 of output absmax (harness gate 2e-2).
"""

import sys

sys.path.insert(0, "/opt/trn_rl_repo")

import numpy as np
import ml_dtypes

B, H, S, D = 2, 12, 2048, 128
NCORES = 8
BH = B * H
HEADS = BH // NCORES  # 3 heads per core
P = 128
NT = S // P           # 16 key tiles
GW = 512              # query-group width (matmul free dim)
G = S // GW           # 4 query groups
TPG = GW // P         # 4 tiles per group
SCALE = float(D) ** -0.5
EXP_BIAS = -1.5       # exp(s*SCALE + EXP_BIAS): keeps E below fp8e4m3 max 240

_PROGRAM = None


def _build_program():
    import concourse.mybir as mybir
    import concourse.tile as tile
    from concourse import bacc

    fp32 = mybir.dt.float32
    fp16 = mybir.dt.float16
    fp8 = mybir.dt.float8e4
    u8 = mybir.dt.uint8
    Exp = mybir.ActivationFunctionType.Exp
    DR = mybir.MatmulPerfMode.DoubleRow

    nc = bacc.Bacc(None)
    qt1 = nc.dram_tensor("qt1", [HEADS, P, S], fp16, kind="ExternalInput")
    kt1 = nc.dram_tensor("kt1", [HEADS, P, S], fp16, kind="ExternalInput")
    qt2 = nc.dram_tensor("qt2", [HEADS, P, S], fp16, kind="ExternalInput")
    kt2 = nc.dram_tensor("kt2", [HEADS, P, S], fp16, kind="ExternalInput")
    v16d = nc.dram_tensor("v16", [HEADS, P, NT, D], fp16, kind="ExternalInput")
    v8d = nc.dram_tensor("v8", [HEADS, P, NT, D], fp8, kind="ExternalInput")
    neglam = nc.dram_tensor("neglam", [P, 1], fp32, kind="ExternalInput")
    identd = nc.dram_tensor("ident", [P, P], fp16, kind="ExternalInput")
    trib = nc.dram_tensor("trib", [P, P], fp16, kind="ExternalInput")
    out = nc.dram_tensor("out", [HEADS, P, S], fp32, kind="ExternalOutput")

    # diag score region packing: region dr covers q-cols [dr*128, 512) of the
    # group. Offsets are PSUM-bank-aligned: a matmul output must not cross a
    # 512-fp32 bank boundary. The inter-region gap columns are never read.
    OFFS = [0, 512, 1024, 1536]
    WIDS = [512, 384, 256, 128]
    DIAGW = 1664  # exp covers [0, 1664): all four regions plus dead gaps

    with tile.TileContext(nc) as tc:
        with (
            tc.tile_pool(name="const", bufs=1) as cpool,
            tc.tile_pool(name="load", bufs=2) as lpool,
            tc.tile_pool(name="et", bufs=3) as epool,
            tc.tile_pool(name="etd", bufs=2) as edpool,
            tc.tile_pool(name="fin", bufs=4) as fpool,
            tc.tile_pool(name="sc", bufs=1, space="PSUM") as spool,
            tc.tile_pool(name="op", bufs=1, space="PSUM") as opool,
            tc.tile_pool(name="up", bufs=1, space="PSUM") as upool,
        ):
            # causal band kill via PE: st[:, band] += I^T @ (-60000 * tri),
            # then exp(SCALE*(s-60000)+bias) == 0. No DVE write into PSUM, so
            # no cross-engine WAW race on the score banks.
            ident = cpool.tile([P, P], fp16)
            nc.sync.dma_start(ident[:], identd[:])
            tribig = cpool.tile([P, P], fp16)
            nc.sync.dma_start(tribig[:], trib[:])
            neglam_s = cpool.tile([P, 1], fp32)
            nc.sync.dma_start(neglam_s[:], neglam[:])
            bias_s = cpool.tile([P, 1], fp32)
            nc.vector.memset(bias_s[:], EXP_BIAS)
            ones8 = cpool.tile([P, 2, P], fp8)
            nc.vector.memset(ones8[:], 1.0)
            ones16 = cpool.tile([P, P], fp16)
            nc.vector.memset(ones16[:], 1.0)

            for h in range(HEADS):
                qk = []
                for name, t in (
                    ("q1", qt1), ("k1", kt1), ("q2", qt2), ("k2", kt2),
                ):
                    ts_ = lpool.tile([P, S], fp16, tag=name)
                    # split loads: first slice covers everything g=0 reads
                    nc.sync.dma_start(ts_[:, 0:GW], t[h][:, 0:GW])
                    nc.sync.dma_start(ts_[:, GW:], t[h][:, GW:])
                    qk.append(ts_)
                v16 = lpool.tile([P, NT, D], fp16, tag="v16")
                nc.sync.dma_start(v16[:, 0:TPG], v16d[h][:, 0:TPG])
                nc.sync.dma_start(v16[:, TPG:], v16d[h][:, TPG:])
                v8 = lpool.tile([P, NT, D], fp8, tag="v8")
                nc.sync.dma_start(v8[:, 0:TPG], v8d[h][:, 0:TPG])
                nc.sync.dma_start(v8[:, TPG:], v8d[h][:, TPG:])

                qs = [qk[0], qk[2]]
                ks = [qk[1], qk[3]]

                for g in range(G):
                    jfull = TPG * g
                    qcols = [qs[pi][:, g * GW : (g + 1) * GW] for pi in range(2)]
                    outp = [
                        opool.tile([P, GW], fp32, tag=f"outp{pi}",
                                   name=f"outp{pi}_{h}_{g}")
                        for pi in range(2)
                    ]
                    sums = [
                        upool.tile([P, GW], fp32, tag=f"sums{pi}",
                                   name=f"sums{pi}_{h}_{g}")
                        for pi in range(2)
                    ]
                    # ---- full key-tile pairs: fp8 E + DoubleRow PV/sums ----
                    for pj in range(jfull // 2):
                        j0 = 2 * pj
                        st = spool.tile([P, 4 * GW], fp32, tag="st")
                        et = epool.tile([P, 4 * GW], fp8, tag="et")
                        for pi in range(2):
                            for dj in range(2):
                                nc.tensor.matmul(
                                    st[:, (2 * pi + dj) * GW : (2 * pi + dj + 1) * GW],
                                    ks[pi][:, (j0 + dj) * P : (j0 + dj + 1) * P],
                                    qcols[pi],
                                    start=True,
                                    stop=True,
                                )
                        nc.scalar.activation(
                            et[:], st[:], Exp, scale=SCALE, bias=bias_s[:]
                        )
                        for pi in range(2):
                            epair = et[:, 2 * pi * GW : (2 * pi + 2) * GW].rearrange(
                                "p (t q) -> p t q", t=2, q=GW
                            )
                            nc.tensor.matmul(
                                sums[pi][:], ones8[:], epair,
                                start=(pj == 0), stop=False,
                                perf_mode=DR, skip_group_check=True,
                            )
                            nc.tensor.matmul(
                                outp[pi][:], v8[:, j0 : j0 + 2, :], epair,
                                start=(pj == 0), stop=False,
                                perf_mode=DR, skip_group_check=True,
                            )
                    # ---- diagonal: fp16, packed [128, 1280] per pass ----
                    for pi in range(2):
                        st = spool.tile([P, 4 * GW], fp32, tag="st")
                        etd = edpool.tile([P, DIAGW], fp16, tag="etd")
                        for dr in range(TPG):
                            j = jfull + dr
                            nc.tensor.matmul(
                                st[:, OFFS[dr] : OFFS[dr] + WIDS[dr]],
                                ks[pi][:, j * P : (j + 1) * P],
                                qs[pi][:, g * GW + dr * P : (g + 1) * GW],
                                start=True,
                                stop=False,
                                skip_group_check=True,
                            )
                            # band kill: first 128 cols of the region
                            nc.tensor.matmul(
                                st[:, OFFS[dr] : OFFS[dr] + P],
                                ident[:],
                                tribig[:],
                                start=False,
                                stop=True,
                                skip_group_check=True,
                            )
                        nc.scalar.activation(
                            etd[:], st[:, 0:DIAGW], Exp,
                            scale=SCALE, bias=bias_s[:],
                        )
                        for dr in range(TPG):
                            j = jfull + dr
                            ecols = etd[:, OFFS[dr] : OFFS[dr] + WIDS[dr]]
                            nc.tensor.matmul(
                                sums[pi][:, dr * P :], ones16[:], ecols,
                                start=(dr == 0 and jfull == 0),
                                stop=(dr == TPG - 1),
                                skip_group_check=True,
                            )
                            nc.tensor.matmul(
                                outp[pi][:, dr * P :], v16[:, j, :], ecols,
                                start=(dr == 0 and jfull == 0),
                                stop=(dr == TPG - 1),
                                skip_group_check=True,
                            )
                    # ---- epilogue: fin = outp0/sums0 - lam*outp1/sums1 ----
                    rcps = []
                    for pi in range(2):
                        rcp = fpool.tile([P, GW], fp32, tag=f"rcp{pi}")
                        nc.vector.reciprocal_approx_fast(rcp[:], sums[pi][:])
                        rcps.append(rcp)
                    t1 = fpool.tile([P, GW], fp32, tag="t1")
                    nc.vector.tensor_mul(t1[:], outp[0][:], rcps[0][:])
                    t2 = fpool.tile([P, GW], fp32, tag="t2")
                    nc.vector.tensor_mul(t2[:], outp[1][:], rcps[1][:])
                    fin = fpool.tile([P, GW], fp32, tag="fin")
                    nc.vector.scalar_tensor_tensor(
                        fin[:], t2[:], neglam_s[:], t1[:],
                        op0=mybir.AluOpType.mult, op1=mybir.AluOpType.add,
                    )
                    nc.sync.dma_start(out[h][:, g * GW : (g + 1) * GW], fin[:])

    nc.compile()
    return nc


def _get_program():
    global _PROGRAM
    if _PROGRAM is None:
        _PROGRAM = _build_program()
    return _PROGRAM


def _make_in_maps(q1, k1, v, q2, k2, lambda_log):
    lam_val = float(np.exp(np.float64(lambda_log.reshape(-1)[0])))
    neglam_np = np.full((P, 1), -lam_val, dtype=np.float32)
    ident_np = np.eye(P, dtype=np.float16)
    # kill-mask for the diagonal band: -60000 where k > q (above causal diag)
    trib_np = np.where(
        np.arange(P)[:, None] > np.arange(P)[None, :], -60000.0, 0.0
    ).astype(np.float16)

    def t(x):  # [BH, S, D] -> [BH, D, S] contiguous fp16
        return np.ascontiguousarray(
            x.reshape(BH, S, D).transpose(0, 2, 1)
        ).astype(np.float16)

    q1t = t(q1)
    q2t = t(q2)
    k1t = t(k1)
    k2t = t(k2)
    # pre-tile V to [BH, p, j, d]: v_s[p, j, d] = V[128 j + p, d]
    vf = np.ascontiguousarray(v.reshape(BH, NT, P, D).transpose(0, 2, 1, 3))
    v16_np = vf.astype(np.float16)
    v8_np = vf.astype(ml_dtypes.float8_e4m3)

    in_maps = []
    for c in range(NCORES):
        sl = slice(c * HEADS, (c + 1) * HEADS)
        in_maps.append(
            {
                "qt1": q1t[sl],
                "kt1": k1t[sl],
                "qt2": q2t[sl],
                "kt2": k2t[sl],
                "v16": v16_np[sl],
                "v8": v8_np[sl],
                "neglam": neglam_np,
                "ident": ident_np,
                "trib": trib_np,
            }
        )
    return in_maps


def _run(q1, k1, v, q2, k2, lambda_log, trace=False):
    from concourse.bass_utils import run_bass_kernel_spmd

    nc = _get_program()
    in_maps = _make_in_maps(q1, k1, v, q2, k2, lambda_log)
    res = run_bass_kernel_spmd(
        nc, in_maps, core_ids=list(range(NCORES)), trace=trace
    )
    parts = [res.results[c]["out"].transpose(0, 2, 1) for c in range(NCORES)]
    full = np.concatenate(parts, axis=0).reshape(B, H, S, D)
    return np.ascontiguousarray(full, dtype=np.float32), res


def kernel(q1, k1, v, q2, k2, lambda_log):
    out, _ = _run(q1, k1, v, q2, k2, lambda_log, trace=False)
    return out


# revision 14
# speedup vs baseline: 1.8036x; 1.8036x over previous
"""Differential attention Trainium2 kernel (Bass/Tile), 8-core SPMD. v3.

reference:
  attn1 = softmax(causal(Q1 K1^T / sqrt(D))) V
  attn2 = softmax(causal(Q2 K2^T / sqrt(D))) V
  out   = attn1 - exp(lambda_log) * attn2
shapes: [B=2, H=12, S=2048, D=128] fp32.

Sharding: B*H = 24 head-batches, 3 per NeuronCore (data/head parallel, no
cross-core comms). Host pre-transposes Q/K to [D, S] fp16; device returns
output d-major ([D, S] per head) and the host transposes back.

Design (per core; v1 baseline was 182 us):
 - Scores in [128, 1024] fp32 PSUM tiles (2 banks), double-buffered (4 banks)
   so the PE fills pair n+1 while ScalarE exps pair n. outp (2) + sums (2)
   use the other 4 banks.
 - Each score tile holds one (j, j+1) key-tile pair of one pass; ONE 1024-col
   ACTIVATE per pair amortizes ScalarE's ~352-cycle per-instruction overhead
   (v1 did 512-col exps: 146 us of ACTIVATE; exp stream floor is ~87 us).
 - Full-tile E is fp8e4m3 (TRN variant, max 240: exp carries a free bias of
   -1.5, a per-column constant that cancels in softmax). PV and row-sum
   matmuls then run as fp8 DoubleRow pair-matmuls (2 key-tiles contracted per
   instruction at 0.5 cyc/row -- measured ~280 ns for a 512-col pair vs
   ~370 ns for ONE fp16 tile) with V and ones stationary in fp8.
   QK stays fp16 (DoubleRow needs a 256-deep contraction; QK's D=128).
 - Diagonal tiles keep fp16 E/V: they contain all short causal rows, where
   fp8 noise does not average out. Their four regions pack into three
   [128, 1024] score allocations (regions never cross a PSUM bank boundary),
   with the dr2/dr3 regions of BOTH passes sharing the third allocation.
 - Causal band kill is done on the PE itself: st[:, band] += I^T @
   (-60000 * tri) accumulated onto the scores, so no other engine writes
   PSUM between QK and exp.
 - Epilogue per (head, group): 1-op reciprocal_approx_fast on each pass's
   broadcast row-sums, two PSUM muls, one scalar_tensor_tensor for
   out = t1 - lam*t2 (lam exact in fp32), DMA out.

Measured end-to-end error ~8e-3 of output absmax (fp8 E+V on long rows);
harness gate is 2e-2.
"""

import sys

sys.path.insert(0, "/opt/trn_rl_repo")

import numpy as np
import ml_dtypes

B, H, S, D = 2, 12, 2048, 128
NCORES = 8
BH = B * H
HEADS = BH // NCORES  # 3 heads per core
P = 128
NT = S // P           # 16 key tiles
GW = 512              # query-group width (psum accumulator free dim)
G = S // GW           # 4 query groups
TPG = GW // P         # 4 tiles per group
SCALE = float(D) ** -0.5
EXP_BIAS = -1.5       # exp(s*SCALE + EXP_BIAS): keeps E below fp8e4m3 max 240
WIDS = [512, 384, 256, 128]  # diag region dr covers q-cols [dr*128, 512)

_PROGRAM = None


def _build_program():
    import concourse.mybir as mybir
    import concourse.tile as tile
    from concourse import bacc

    fp32 = mybir.dt.float32
    fp16 = mybir.dt.float16
    fp8 = mybir.dt.float8e4
    Exp = mybir.ActivationFunctionType.Exp
    DR = mybir.MatmulPerfMode.DoubleRow

    nc = bacc.Bacc(None)
    qt1 = nc.dram_tensor("qt1", [HEADS, P, S], fp16, kind="ExternalInput")
    kt1 = nc.dram_tensor("kt1", [HEADS, P, S], fp16, kind="ExternalInput")
    qt2 = nc.dram_tensor("qt2", [HEADS, P, S], fp16, kind="ExternalInput")
    kt2 = nc.dram_tensor("kt2", [HEADS, P, S], fp16, kind="ExternalInput")
    v16d = nc.dram_tensor("v16", [HEADS, P, NT, D], fp16, kind="ExternalInput")
    v8d = nc.dram_tensor("v8", [HEADS, P, NT, D], fp8, kind="ExternalInput")
    neglam = nc.dram_tensor("neglam", [P, 1], fp32, kind="ExternalInput")
    identd = nc.dram_tensor("ident", [P, P], fp16, kind="ExternalInput")
    trib = nc.dram_tensor("trib", [P, P], fp16, kind="ExternalInput")
    out = nc.dram_tensor("out", [HEADS, P, S], fp32, kind="ExternalOutput")

    with tile.TileContext(nc) as tc:
        with (
            tc.tile_pool(name="const", bufs=1) as cpool,
            tc.tile_pool(name="load", bufs=2) as lpool,
            tc.tile_pool(name="et", bufs=4) as epool,
            tc.tile_pool(name="etd", bufs=2) as edpool,
            tc.tile_pool(name="fin", bufs=4) as fpool,
            tc.tile_pool(name="sc", bufs=2, space="PSUM") as spool,
            tc.tile_pool(name="op", bufs=1, space="PSUM") as opool,
            tc.tile_pool(name="up", bufs=1, space="PSUM") as upool,
        ):
            ident = cpool.tile([P, P], fp16)
            nc.sync.dma_start(ident[:], identd[:])
            tribig = cpool.tile([P, P], fp16)
            nc.sync.dma_start(tribig[:], trib[:])
            neglam_s = cpool.tile([P, 1], fp32)
            nc.sync.dma_start(neglam_s[:], neglam[:])
            bias_s = cpool.tile([P, 1], fp32)
            nc.vector.memset(bias_s[:], EXP_BIAS)
            ones8 = cpool.tile([P, 2, P], fp8)
            nc.vector.memset(ones8[:], 1.0)
            ones16 = cpool.tile([P, P], fp16)
            nc.vector.memset(ones16[:], 1.0)

            def band_kill(st, off):
                # st[:, off:off+128] += -60000 where k > q (PE-side causal mask;
                # joins the score accumulation group, so pair each QK matmul
                # with stop=False and this with stop=True)
                nc.tensor.matmul(
                    st[:, off : off + P], ident[:], tribig[:],
                    start=False, stop=True, skip_group_check=True,
                )

            for h in range(HEADS):
                qk = []
                for name, t in (
                    ("q1", qt1), ("k1", kt1), ("q2", qt2), ("k2", kt2),
                ):
                    ts_ = lpool.tile([P, S], fp16, tag=name)
                    # split loads: first slice covers everything g=0 reads
                    nc.sync.dma_start(ts_[:, 0:GW], t[h][:, 0:GW])
                    nc.sync.dma_start(ts_[:, GW:], t[h][:, GW:])
                    qk.append(ts_)
                v16 = lpool.tile([P, NT, D], fp16, tag="v16")
                nc.sync.dma_start(v16[:, 0:TPG], v16d[h][:, 0:TPG])
                nc.sync.dma_start(v16[:, TPG:], v16d[h][:, TPG:])
                v8 = lpool.tile([P, NT, D], fp8, tag="v8")
                nc.sync.dma_start(v8[:, 0:TPG], v8d[h][:, 0:TPG])
                nc.sync.dma_start(v8[:, TPG:], v8d[h][:, TPG:])

                qs = [qk[0], qk[2]]
                ks = [qk[1], qk[3]]

                for g in range(G):
                    jfull = TPG * g
                    qcols = [qs[pi][:, g * GW : (g + 1) * GW] for pi in range(2)]
                    outp = [
                        opool.tile([P, GW], fp32, tag=f"outp{pi}",
                                   name=f"outp{pi}_{h}_{g}")
                        for pi in range(2)
                    ]
                    sums = [
                        upool.tile([P, GW], fp32, tag=f"sums{pi}",
                                   name=f"sums{pi}_{h}_{g}")
                        for pi in range(2)
                    ]

                    # ---- full key-tile pairs: fp8 E + DoubleRow PV/sums ----
                    for pj in range(jfull // 2):
                        j0 = 2 * pj
                        for pi in range(2):
                            st = spool.tile([P, 1024], fp32, tag="st")
                            et = epool.tile([P, 1024], fp8, tag="et")
                            for dj in range(2):
                                nc.tensor.matmul(
                                    st[:, dj * GW : (dj + 1) * GW],
                                    ks[pi][:, (j0 + dj) * P : (j0 + dj + 1) * P],
                                    qcols[pi],
                                    start=True,
                                    stop=True,
                                )
                            nc.scalar.activation(
                                et[:], st[:], Exp, scale=SCALE, bias=bias_s[:]
                            )
                            epair = et[:].rearrange("p (t q) -> p t q", t=2, q=GW)
                            nc.tensor.matmul(
                                sums[pi][:], ones8[:], epair,
                                start=(pj == 0), stop=False,
                                perf_mode=DR, skip_group_check=True,
                            )
                            nc.tensor.matmul(
                                outp[pi][:], v8[:, j0 : j0 + 2, :], epair,
                                start=(pj == 0), stop=False,
                                perf_mode=DR, skip_group_check=True,
                            )

                    # ---- diagonal: fp16, 3 allocations per group ----
                    # A/B (per pass): dr0 at [0:512], dr1 at [512:896]
                    # C (shared):   p0dr2 [0:256], p1dr2 [256:512],
                    #               p0dr3 [512:640], p1dr3 [640:768]
                    etds = []
                    for pi in range(2):
                        st = spool.tile([P, 1024], fp32, tag="st")
                        etd = edpool.tile([P, 896], fp16, tag=f"etd{pi}")
                        for dr, off in ((0, 0), (1, 512)):
                            j = jfull + dr
                            nc.tensor.matmul(
                                st[:, off : off + WIDS[dr]],
                                ks[pi][:, j * P : (j + 1) * P],
                                qs[pi][:, g * GW + dr * P : (g + 1) * GW],
                                start=True, stop=False, skip_group_check=True,
                            )
                            band_kill(st, off)
                        nc.scalar.activation(
                            etd[:], st[:, 0:896], Exp,
                            scale=SCALE, bias=bias_s[:],
                        )
                        etds.append(etd)
                    stc = spool.tile([P, 1024], fp32, tag="st")
                    etc = edpool.tile([P, 768], fp16, tag="etdc")
                    for pi in range(2):
                        for dr, off in ((2, 256 * pi), (3, 512 + 128 * pi)):
                            j = jfull + dr
                            nc.tensor.matmul(
                                stc[:, off : off + WIDS[dr]],
                                ks[pi][:, j * P : (j + 1) * P],
                                qs[pi][:, g * GW + dr * P : (g + 1) * GW],
                                start=True, stop=False, skip_group_check=True,
                            )
                            band_kill(stc, off)
                    nc.scalar.activation(
                        etc[:], stc[:, 0:768], Exp,
                        scale=SCALE, bias=bias_s[:],
                    )
                    for pi in range(2):
                        ecols = {
                            0: etds[pi][:, 0:512],
                            1: etds[pi][:, 512:896],
                            2: etc[:, 256 * pi : 256 * pi + 256],
                            3: etc[:, 512 + 128 * pi : 640 + 128 * pi],
                        }
                        for dr in range(TPG):
                            j = jfull + dr
                            nc.tensor.matmul(
                                sums[pi][:, dr * P :], ones16[:], ecols[dr],
                                start=(dr == 0 and jfull == 0),
                                stop=(dr == TPG - 1),
                                skip_group_check=True,
                            )
                            nc.tensor.matmul(
                                outp[pi][:, dr * P :], v16[:, j, :], ecols[dr],
                                start=(dr == 0 and jfull == 0),
                                stop=(dr == TPG - 1),
                                skip_group_check=True,
                            )

                    # ---- epilogue: fin = outp0/sums0 - lam*outp1/sums1 ----
                    rcps = []
                    for pi in range(2):
                        rcp = fpool.tile([P, GW], fp32, tag=f"rcp{pi}")
                        nc.vector.reciprocal_approx_fast(rcp[:], sums[pi][:])
                        rcps.append(rcp)
                    t1 = fpool.tile([P, GW], fp32, tag="t1")
                    nc.vector.tensor_mul(t1[:], outp[0][:], rcps[0][:])
                    t2 = fpool.tile([P, GW], fp32, tag="t2")
                    nc.vector.tensor_mul(t2[:], outp[1][:], rcps[1][:])
                    fin = fpool.tile([P, GW], fp32, tag="fin")
                    nc.vector.scalar_tensor_tensor(
                        fin[:], t2[:], neglam_s[:], t1[:],
                        op0=mybir.AluOpType.mult, op1=mybir.AluOpType.add,
                    )
                    nc.sync.dma_start(out[h][:, g * GW : (g + 1) * GW], fin[:])

    nc.compile()
    return nc


def _get_program():
    global _PROGRAM
    if _PROGRAM is None:
        _PROGRAM = _build_program()
    return _PROGRAM


def _make_in_maps(q1, k1, v, q2, k2, lambda_log):
    lam_val = float(np.exp(np.float64(lambda_log.reshape(-1)[0])))
    neglam_np = np.full((P, 1), -lam_val, dtype=np.float32)
    ident_np = np.eye(P, dtype=np.float16)
    # kill-mask for the diagonal band: -60000 where k > q (above causal diag)
    trib_np = np.where(
        np.arange(P)[:, None] > np.arange(P)[None, :], -60000.0, 0.0
    ).astype(np.float16)

    def t(x):  # [BH, S, D] -> [BH, D, S] contiguous fp16
        return np.ascontiguousarray(
            x.reshape(BH, S, D).transpose(0, 2, 1)
        ).astype(np.float16)

    q1t = t(q1)
    q2t = t(q2)
    k1t = t(k1)
    k2t = t(k2)
    # pre-tile V to [BH, p, j, d]: v_s[p, j, d] = V[128 j + p, d]
    vf = np.ascontiguousarray(v.reshape(BH, NT, P, D).transpose(0, 2, 1, 3))
    v16_np = vf.astype(np.float16)
    v8_np = vf.astype(ml_dtypes.float8_e4m3)

    in_maps = []
    for c in range(NCORES):
        sl = slice(c * HEADS, (c + 1) * HEADS)
        in_maps.append(
            {
                "qt1": q1t[sl],
                "kt1": k1t[sl],
                "qt2": q2t[sl],
                "kt2": k2t[sl],
                "v16": v16_np[sl],
                "v8": v8_np[sl],
                "neglam": neglam_np,
                "ident": ident_np,
                "trib": trib_np,
            }
        )
    return in_maps


def _run(q1, k1, v, q2, k2, lambda_log, trace=False):
    from concourse.bass_utils import run_bass_kernel_spmd

    nc = _get_program()
    in_maps = _make_in_maps(q1, k1, v, q2, k2, lambda_log)
    res = run_bass_kernel_spmd(
        nc, in_maps, core_ids=list(range(NCORES)), trace=trace
    )
    parts = [res.results[c]["out"].transpose(0, 2, 1) for c in range(NCORES)]
    full = np.concatenate(parts, axis=0).reshape(B, H, S, D)
    return np.ascontiguousarray(full, dtype=np.float32), res


def kernel(q1, k1, v, q2, k2, lambda_log):
    out, _ = _run(q1, k1, v, q2, k2, lambda_log, trace=False)
    return out


# revision 31
# speedup vs baseline: 1.8148x; 1.0062x over previous
"""Differential attention Trainium2 kernel (Bass/Tile), 8-core SPMD. v3.

reference:
  attn1 = softmax(causal(Q1 K1^T / sqrt(D))) V
  attn2 = softmax(causal(Q2 K2^T / sqrt(D))) V
  out   = attn1 - exp(lambda_log) * attn2
shapes: [B=2, H=12, S=2048, D=128] fp32.

Sharding: B*H = 24 head-batches, 3 per NeuronCore (data/head parallel, no
cross-core comms). Host pre-transposes Q/K to [D, S] fp16; device returns
output d-major ([D, S] per head) and the host transposes back.

Design (per core; v1 baseline was 182 us):
 - Scores in [128, 1024] fp32 PSUM tiles (2 banks), double-buffered (4 banks)
   so the PE fills pair n+1 while ScalarE exps pair n. outp (2) + sums (2)
   use the other 4 banks.
 - Each score tile holds one (j, j+1) key-tile pair of one pass; ONE 1024-col
   ACTIVATE per pair amortizes ScalarE's ~352-cycle per-instruction overhead
   (v1 did 512-col exps: 146 us of ACTIVATE; exp stream floor is ~87 us).
 - Full-tile E is fp8e4m3 (TRN variant, max 240: exp carries a free bias of
   -1.5, a per-column constant that cancels in softmax). PV and row-sum
   matmuls then run as fp8 DoubleRow pair-matmuls (2 key-tiles contracted per
   instruction at 0.5 cyc/row -- measured ~280 ns for a 512-col pair vs
   ~370 ns for ONE fp16 tile) with V and ones stationary in fp8.
   QK stays fp16 (DoubleRow needs a 256-deep contraction; QK's D=128).
 - Diagonal tiles keep fp16 E/V: they contain all short causal rows, where
   fp8 noise does not average out. Their four regions pack into three
   [128, 1024] score allocations (regions never cross a PSUM bank boundary),
   with the dr2/dr3 regions of BOTH passes sharing the third allocation.
 - Causal band kill is done on the PE itself: st[:, band] += I^T @
   (-60000 * tri) accumulated onto the scores, so no other engine writes
   PSUM between QK and exp.
 - Epilogue per (head, group): 1-op reciprocal_approx_fast on each pass's
   broadcast row-sums, two PSUM muls, one scalar_tensor_tensor for
   out = t1 - lam*t2 (lam exact in fp32), DMA out.

Measured end-to-end error ~8e-3 of output absmax (fp8 E+V on long rows);
harness gate is 2e-2.
"""

import sys

sys.path.insert(0, "/opt/trn_rl_repo")

import numpy as np
import ml_dtypes

B, H, S, D = 2, 12, 2048, 128
NCORES = 8
BH = B * H
HEADS = BH // NCORES  # 3 heads per core
P = 128
NT = S // P           # 16 key tiles
GW = 512              # query-group width (psum accumulator free dim)
G = S // GW           # 4 query groups
TPG = GW // P         # 4 tiles per group
SCALE = float(D) ** -0.5
EXP_BIAS = -1.5       # exp(s*SCALE + EXP_BIAS): keeps E below fp8e4m3 max 240
WIDS = [512, 384, 256, 128]  # diag region dr covers q-cols [dr*128, 512)

_PROGRAM = None


def _build_program():
    import concourse.mybir as mybir
    import concourse.tile as tile
    from concourse import bacc

    fp32 = mybir.dt.float32
    fp16 = mybir.dt.float16
    fp8 = mybir.dt.float8e4
    Exp = mybir.ActivationFunctionType.Exp
    DR = mybir.MatmulPerfMode.DoubleRow

    nc = bacc.Bacc(None)
    # q/k packed as [head, partition, tensor(q1,k1,q2,k2), cols]: the first
    # GW columns and the tail live in separate tensors so each head's
    # critical first-slice load is ONE dma_start with 4KB-contiguous rows
    # (separate per-tensor loads moved as ~1KB packets at ~60 GB/s)
    qkfd = nc.dram_tensor("qkf", [HEADS, P, 4, GW], fp16, kind="ExternalInput")
    qktd = nc.dram_tensor("qkt", [HEADS, P, 4, S - GW], fp16,
                          kind="ExternalInput")
    v16d = nc.dram_tensor("v16", [HEADS, P, NT, D], fp16, kind="ExternalInput")
    v8d = nc.dram_tensor("v8", [HEADS, P, NT, D], fp8, kind="ExternalInput")
    neglam = nc.dram_tensor("neglam", [P, 1], fp32, kind="ExternalInput")
    identd = nc.dram_tensor("ident", [P, P], fp16, kind="ExternalInput")
    trib = nc.dram_tensor("trib", [P, P], fp16, kind="ExternalInput")
    bandc0d = nc.dram_tensor("bandc0", [P, 384], fp16, kind="ExternalInput")
    bandc1d = nc.dram_tensor("bandc1", [P, 256], fp16, kind="ExternalInput")
    out = nc.dram_tensor("out", [HEADS, P, S], fp32, kind="ExternalOutput")

    with tile.TileContext(nc) as tc:
        with (
            tc.tile_pool(name="const", bufs=1) as cpool,
            tc.tile_pool(name="load", bufs=2) as lpool,
            tc.tile_pool(name="et", bufs=8) as epool,
            tc.tile_pool(name="etd", bufs=4) as edpool,
            tc.tile_pool(name="fin", bufs=4) as fpool,
            tc.tile_pool(name="sc", bufs=2, space="PSUM") as spool,
            tc.tile_pool(name="op", bufs=1, space="PSUM") as opool,
            tc.tile_pool(name="up", bufs=1, space="PSUM") as upool,
        ):
            def load_head(h):
                qkf = lpool.tile([P, 4, GW], fp16, tag="qkf",
                                 name=f"qkf_{h}")
                nc.sync.dma_start(qkf[:], qkfd[h])
                v16 = lpool.tile([P, NT, D], fp16, tag="v16",
                                 name=f"v16_{h}")
                v8 = lpool.tile([P, NT, D], fp8, tag="v8", name=f"v8_{h}")
                nc.sync.dma_start(v16[:, 0:TPG], v16d[h][:, 0:TPG])
                nc.sync.dma_start(v8[:, 0:TPG], v8d[h][:, 0:TPG])
                qkt = lpool.tile([P, 4, S - GW], fp16, tag="qkt",
                                 name=f"qkt_{h}")
                nc.sync.dma_start(qkt[:], qktd[h])
                nc.sync.dma_start(v16[:, TPG:], v16d[h][:, TPG:])
                nc.sync.dma_start(v8[:, TPG:], v8d[h][:, TPG:])
                return qkf, qkt, v16, v8

            # head 0's critical loads enqueue before everything else; the
            # five tiny const DMAs would otherwise delay them ~3.5us of
            # Sync descriptor-gen
            head0_tiles = load_head(0)

            ident = cpool.tile([P, P], fp16)
            nc.sync.dma_start(ident[:], identd[:])
            tribig = cpool.tile([P, P], fp16)
            nc.sync.dma_start(tribig[:], trib[:])
            neglam_s = cpool.tile([P, 1], fp32)
            nc.sync.dma_start(neglam_s[:], neglam[:])
            bias_s = cpool.tile([P, 1], fp32)
            nc.vector.memset(bias_s[:], EXP_BIAS)
            ones8 = cpool.tile([P, 2, P], fp8)
            nc.vector.memset(ones8[:], 1.0)
            ones16 = cpool.tile([P, P], fp16)
            nc.vector.memset(ones16[:], 1.0)
            bandc0 = cpool.tile([P, 384], fp16)
            nc.sync.dma_start(bandc0[:], bandc0d[:])
            bandc1 = cpool.tile([P, 256], fp16)
            nc.sync.dma_start(bandc1[:], bandc1d[:])
            # trigger the exp ACT-table load (~2.7us) during the input DMAs
            # instead of on the first real exp
            dummy = cpool.tile([P, 1], fp32)
            nc.scalar.activation(dummy[:], bias_s[:], Exp)

            def band_kill(st, off):
                # st[:, off:off+128] += -60000 where k > q (PE-side causal
                # mask accumulated onto the scores)
                nc.tensor.matmul(
                    st[:, off : off + P], ident[:], tribig[:],
                    start=False, stop=True, skip_group_check=True,
                )

            for h in range(HEADS):
                qkf, qkt, v16, v8 = head0_tiles if h == 0 else load_head(h)

                def qk_cols(ti, c0, c1):
                    # columns [c0, c1) of packed tensor ti (0=q1,1=k1,2=q2,3=k2)
                    if c1 <= GW:
                        return qkf[:, ti, c0:c1]
                    assert c0 >= GW
                    return qkt[:, ti, c0 - GW : c1 - GW]

                for g in range(G):
                    jfull = TPG * g
                    qcols = [qk_cols(2 * pi, g * GW, (g + 1) * GW)
                             for pi in range(2)]
                    # both passes' accumulators in one 2-bank tile each, so
                    # the epilogue drains them with single wide DVE ops
                    outp_t = opool.tile([P, 2 * GW], fp32, tag="outp",
                                        name=f"outp_{h}_{g}")
                    sums_t = upool.tile([P, 2 * GW], fp32, tag="sums",
                                        name=f"sums_{h}_{g}")

                    # ---- full key-tile pairs: fp8 E + DoubleRow PV/sums ----
                    for pj in range(jfull // 2):
                        j0 = 2 * pj
                        for pi in range(2):
                            st = spool.tile([P, 1024], fp32, tag="st")
                            et = epool.tile([P, 1024], fp8, tag="et")
                            for dj in range(2):
                                nc.tensor.matmul(
                                    st[:, dj * GW : (dj + 1) * GW],
                                    qk_cols(2 * pi + 1, (j0 + dj) * P,
                                            (j0 + dj + 1) * P),
                                    qcols[pi],
                                    start=True,
                                    stop=True,
                                )
                            nc.scalar.activation(
                                et[:], st[:], Exp, scale=SCALE, bias=bias_s[:]
                            )
                            epair = et[:].rearrange("p (t q) -> p t q", t=2, q=GW)
                            nc.tensor.matmul(
                                sums_t[:, pi * GW : (pi + 1) * GW],
                                ones8[:], epair,
                                start=(pj == 0), stop=False,
                                perf_mode=DR, skip_group_check=True,
                            )
                            nc.tensor.matmul(
                                outp_t[:, pi * GW : (pi + 1) * GW],
                                v8[:, j0 : j0 + 2, :], epair,
                                start=(pj == 0), stop=False,
                                perf_mode=DR, skip_group_check=True,
                            )

                    # ---- diagonal: fp16, 3 allocations per group ----
                    # A/B (per pass): dr0 at [0:512], dr1 at [512:896]
                    # C (shared):   p0dr2 [0:256], p1dr2 [256:512],
                    #               p0dr3 [512:640], p1dr3 [640:768]
                    etds = []
                    for pi in range(2):
                        st = spool.tile([P, 1024], fp32, tag="st")
                        etd = edpool.tile([P, 896], fp16, tag=f"etd{pi}")
                        for dr, off in ((0, 0), (1, 512)):
                            j = jfull + dr
                            nc.tensor.matmul(
                                st[:, off : off + WIDS[dr]],
                                qk_cols(2 * pi + 1, j * P, (j + 1) * P),
                                qk_cols(2 * pi, g * GW + dr * P, (g + 1) * GW),
                                start=True, stop=False, skip_group_check=True,
                            )
                            band_kill(st, off)
                        nc.scalar.activation(
                            etd[:], st[:, 0:896], Exp,
                            scale=SCALE, bias=bias_s[:],
                        )
                        etds.append(etd)
                    stc = spool.tile([P, 1024], fp32, tag="st")
                    etc = edpool.tile([P, 768], fp16, tag="etdc")
                    for pi in range(2):
                        for dr, off in ((2, 256 * pi), (3, 512 + 128 * pi)):
                            j = jfull + dr
                            nc.tensor.matmul(
                                stc[:, off : off + WIDS[dr]],
                                qk_cols(2 * pi + 1, j * P, (j + 1) * P),
                                qk_cols(2 * pi, g * GW + dr * P, (g + 1) * GW),
                                start=True, stop=False, skip_group_check=True,
                            )
                            band_kill(stc, off)
                    nc.scalar.activation(
                        etc[:], stc[:, 0:768], Exp,
                        scale=SCALE, bias=bias_s[:],
                    )
                    for pi in range(2):
                        ecols = {
                            0: etds[pi][:, 0:512],
                            1: etds[pi][:, 512:896],
                            2: etc[:, 256 * pi : 256 * pi + 256],
                            3: etc[:, 512 + 128 * pi : 640 + 128 * pi],
                        }
                        for dr in range(TPG):
                            j = jfull + dr
                            nc.tensor.matmul(
                                sums_t[:, pi * GW + dr * P : (pi + 1) * GW],
                                ones16[:], ecols[dr],
                                start=(dr == 0 and jfull == 0),
                                stop=(dr == TPG - 1),
                                skip_group_check=True,
                            )
                            nc.tensor.matmul(
                                outp_t[:, pi * GW + dr * P : (pi + 1) * GW],
                                v16[:, j, :], ecols[dr],
                                start=(dr == 0 and jfull == 0),
                                stop=(dr == TPG - 1),
                                skip_group_check=True,
                            )

                    # ---- epilogue: fin = outp0/sums0 - lam*outp1/sums1 ----
                    # one wide reciprocal + one wide mul drain both passes'
                    # PSUM accumulators, minimizing how long the next group's
                    # first matmuls (start=True writers) stay blocked
                    rcp = fpool.tile([P, 2 * GW], fp32, tag="rcp")
                    nc.vector.reciprocal_approx_fast(rcp[:], sums_t[:])
                    t12 = fpool.tile([P, 2 * GW], fp32, tag="t12")
                    nc.vector.tensor_mul(t12[:], outp_t[:], rcp[:])
                    fin = fpool.tile([P, GW], fp32, tag="fin")
                    nc.vector.scalar_tensor_tensor(
                        fin[:], t12[:, GW:], neglam_s[:], t12[:, 0:GW],
                        op0=mybir.AluOpType.mult, op1=mybir.AluOpType.add,
                    )
                    nc.sync.dma_start(out[h][:, g * GW : (g + 1) * GW], fin[:])

    nc.compile()
    return nc


def _get_program():
    global _PROGRAM
    if _PROGRAM is None:
        _PROGRAM = _build_program()
    return _PROGRAM


def _make_in_maps(q1, k1, v, q2, k2, lambda_log):
    lam_val = float(np.exp(np.float64(lambda_log.reshape(-1)[0])))
    neglam_np = np.full((P, 1), -lam_val, dtype=np.float32)
    ident_np = np.eye(P, dtype=np.float16)
    # kill-mask for the diagonal band: -60000 where k > q (above causal diag)
    trib_np = np.where(
        np.arange(P)[:, None] > np.arange(P)[None, :], -60000.0, 0.0
    ).astype(np.float16)
    # combined band patterns for the shared dr2/dr3 diag allocation:
    # bank0 [0:384]: bands at 0 (p0dr2) and 256 (p1dr2);
    # bank1 [512:768]: bands at 512 (p0dr3) and 640 (p1dr3)
    bandc0_np = np.zeros((P, 384), dtype=np.float16)
    bandc0_np[:, 0:P] = trib_np
    bandc0_np[:, 256:384] = trib_np
    bandc1_np = np.zeros((P, 256), dtype=np.float16)
    bandc1_np[:, 0:P] = trib_np
    bandc1_np[:, P:256] = trib_np

    def t(x):  # [BH, S, D] -> [BH, D, S] contiguous fp16
        return np.ascontiguousarray(
            x.reshape(BH, S, D).transpose(0, 2, 1)
        ).astype(np.float16)

    q1t = t(q1)
    q2t = t(q2)
    k1t = t(k1)
    k2t = t(k2)
    qk4 = np.stack([q1t, k1t, q2t, k2t], axis=2)  # [BH, P, 4, S]
    qkf_np = np.ascontiguousarray(qk4[:, :, :, 0:GW])
    qkt_np = np.ascontiguousarray(qk4[:, :, :, GW:])
    # pre-tile V to [BH, p, j, d]: v_s[p, j, d] = V[128 j + p, d]
    vf = np.ascontiguousarray(v.reshape(BH, NT, P, D).transpose(0, 2, 1, 3))
    v16_np = vf.astype(np.float16)
    v8_np = vf.astype(ml_dtypes.float8_e4m3)

    in_maps = []
    for c in range(NCORES):
        sl = slice(c * HEADS, (c + 1) * HEADS)
        in_maps.append(
            {
                "qkf": qkf_np[sl],
                "qkt": qkt_np[sl],
                "v16": v16_np[sl],
                "v8": v8_np[sl],
                "neglam": neglam_np,
                "ident": ident_np,
                "trib": trib_np,
                "bandc0": bandc0_np,
                "bandc1": bandc1_np,
            }
        )
    return in_maps


def _run(q1, k1, v, q2, k2, lambda_log, trace=False):
    from concourse.bass_utils import run_bass_kernel_spmd

    nc = _get_program()
    in_maps = _make_in_maps(q1, k1, v, q2, k2, lambda_log)
    res = run_bass_kernel_spmd(
        nc, in_maps, core_ids=list(range(NCORES)), trace=trace
    )
    parts = [res.results[c]["out"].transpose(0, 2, 1) for c in range(NCORES)]
    full = np.concatenate(parts, axis=0).reshape(B, H, S, D)
    return np.ascontiguousarray(full, dtype=np.float32), res


def kernel(q1, k1, v, q2, k2, lambda_log):
    out, _ = _run(q1, k1, v, q2, k2, lambda_log, trace=False)
    return out


# revision 32
# speedup vs baseline: 1.8751x; 1.0332x over previous
"""Differential attention Trainium2 kernel (Bass/Tile), 8-core SPMD. v3.

reference:
  attn1 = softmax(causal(Q1 K1^T / sqrt(D))) V
  attn2 = softmax(causal(Q2 K2^T / sqrt(D))) V
  out   = attn1 - exp(lambda_log) * attn2
shapes: [B=2, H=12, S=2048, D=128] fp32.

Sharding: B*H = 24 head-batches, 3 per NeuronCore (data/head parallel, no
cross-core comms). Host pre-transposes Q/K to [D, S] fp16; device returns
output d-major ([D, S] per head) and the host transposes back.

Design (per core; v1 baseline was 182 us):
 - Scores in [128, 1024] fp32 PSUM tiles (2 banks), double-buffered (4 banks)
   so the PE fills pair n+1 while ScalarE exps pair n. outp (2) + sums (2)
   use the other 4 banks.
 - Each score tile holds one (j, j+1) key-tile pair of one pass; ONE 1024-col
   ACTIVATE per pair amortizes ScalarE's ~352-cycle per-instruction overhead
   (v1 did 512-col exps: 146 us of ACTIVATE; exp stream floor is ~87 us).
 - Full-tile E is fp8e4m3 (TRN variant, max 240: exp carries a free bias of
   -1.5, a per-column constant that cancels in softmax). PV and row-sum
   matmuls then run as fp8 DoubleRow pair-matmuls (2 key-tiles contracted per
   instruction at 0.5 cyc/row -- measured ~280 ns for a 512-col pair vs
   ~370 ns for ONE fp16 tile) with V and ones stationary in fp8.
   QK stays fp16 (DoubleRow needs a 256-deep contraction; QK's D=128).
 - Diagonal tiles keep fp16 E/V: they contain all short causal rows, where
   fp8 noise does not average out. Their four regions pack into three
   [128, 1024] score allocations (regions never cross a PSUM bank boundary),
   with the dr2/dr3 regions of BOTH passes sharing the third allocation.
 - Causal band kill is done on the PE itself: st[:, band] += I^T @
   (-60000 * tri) accumulated onto the scores, so no other engine writes
   PSUM between QK and exp.
 - Epilogue per (head, group): 1-op reciprocal_approx_fast on each pass's
   broadcast row-sums, two PSUM muls, one scalar_tensor_tensor for
   out = t1 - lam*t2 (lam exact in fp32), DMA out.

Measured end-to-end error ~8e-3 of output absmax (fp8 E+V on long rows);
harness gate is 2e-2.
"""

import sys

sys.path.insert(0, "/opt/trn_rl_repo")

import numpy as np
import ml_dtypes

B, H, S, D = 2, 12, 2048, 128
NCORES = 8
BH = B * H
HEADS = BH // NCORES  # 3 heads per core
P = 128
NT = S // P           # 16 key tiles
GW = 512              # query-group width (psum accumulator free dim)
G = S // GW           # 4 query groups
TPG = GW // P         # 4 tiles per group
SCALE = float(D) ** -0.5
EXP_BIAS = -1.5       # exp(s*SCALE + EXP_BIAS): keeps E below fp8e4m3 max 240
WIDS = [512, 384, 256, 128]  # diag region dr covers q-cols [dr*128, 512)

_PROGRAM = None


def _build_program():
    import concourse.mybir as mybir
    import concourse.tile as tile
    from concourse import bacc

    fp32 = mybir.dt.float32
    fp16 = mybir.dt.float16
    fp8 = mybir.dt.float8e4
    Exp = mybir.ActivationFunctionType.Exp
    DR = mybir.MatmulPerfMode.DoubleRow

    nc = bacc.Bacc(None)
    # q/k packed as [head, partition, tensor(q1,k1,q2,k2), cols]: the first
    # GW columns and the tail live in separate tensors so each head's
    # critical first-slice load is ONE dma_start with 4KB-contiguous rows
    # (separate per-tensor loads moved as ~1KB packets at ~60 GB/s)
    qkfd = nc.dram_tensor("qkf", [HEADS, P, 4, GW], fp16, kind="ExternalInput")
    qktd = nc.dram_tensor("qkt", [HEADS, P, 4, S - GW], fp16,
                          kind="ExternalInput")
    v16d = nc.dram_tensor("v16", [HEADS, P, NT, D], fp16, kind="ExternalInput")
    v8d = nc.dram_tensor("v8", [HEADS, P, NT, D], fp8, kind="ExternalInput")
    neglam = nc.dram_tensor("neglam", [P, 1], fp32, kind="ExternalInput")
    identd = nc.dram_tensor("ident", [P, P], fp16, kind="ExternalInput")
    trib = nc.dram_tensor("trib", [P, P], fp16, kind="ExternalInput")
    bandc0d = nc.dram_tensor("bandc0", [P, 384], fp16, kind="ExternalInput")
    bandc1d = nc.dram_tensor("bandc1", [P, 256], fp16, kind="ExternalInput")
    out = nc.dram_tensor("out", [HEADS, P, S], fp32, kind="ExternalOutput")

    with tile.TileContext(nc) as tc:
        with (
            tc.tile_pool(name="const", bufs=1) as cpool,
            tc.tile_pool(name="load", bufs=2) as lpool,
            tc.tile_pool(name="et", bufs=8) as epool,
            tc.tile_pool(name="etd", bufs=4) as edpool,
            tc.tile_pool(name="fin", bufs=4) as fpool,
            tc.tile_pool(name="sc", bufs=2, space="PSUM") as spool,
            tc.tile_pool(name="op", bufs=1, space="PSUM") as opool,
            tc.tile_pool(name="up", bufs=1, space="PSUM") as upool,
        ):
            ident = cpool.tile([P, P], fp16)
            nc.sync.dma_start(ident[:], identd[:])
            tribig = cpool.tile([P, P], fp16)
            nc.sync.dma_start(tribig[:], trib[:])
            neglam_s = cpool.tile([P, 1], fp32)
            nc.sync.dma_start(neglam_s[:], neglam[:])
            bias_s = cpool.tile([P, 1], fp32)
            nc.vector.memset(bias_s[:], EXP_BIAS)
            ones8 = cpool.tile([P, 2, P], fp8)
            nc.vector.memset(ones8[:], 1.0)
            ones16 = cpool.tile([P, P], fp16)
            nc.vector.memset(ones16[:], 1.0)
            bandc0 = cpool.tile([P, 384], fp16)
            nc.sync.dma_start(bandc0[:], bandc0d[:])
            bandc1 = cpool.tile([P, 256], fp16)
            nc.sync.dma_start(bandc1[:], bandc1d[:])
            # trigger the exp ACT-table load (~2.7us) during the input DMAs
            # instead of on the first real exp
            dummy = cpool.tile([P, 1], fp32)
            nc.scalar.activation(dummy[:], bias_s[:], Exp)

            def band_kill(st, off):
                # st[:, off:off+128] += -60000 where k > q (PE-side causal
                # mask accumulated onto the scores)
                nc.tensor.matmul(
                    st[:, off : off + P], ident[:], tribig[:],
                    start=False, stop=True, skip_group_check=True,
                )

            for h in range(HEADS):
                # everything g=0 reads (and nothing else) arrives in two
                # fat-packet DMAs before the 7.5 MB of tails enqueues
                qkf = lpool.tile([P, 4, GW], fp16, tag="qkf")
                qkt = lpool.tile([P, 4, S - GW], fp16, tag="qkt")
                v16 = lpool.tile([P, NT, D], fp16, tag="v16")
                v8 = lpool.tile([P, NT, D], fp8, tag="v8")
                nc.sync.dma_start(qkf[:], qkfd[h])
                nc.sync.dma_start(v16[:, 0:TPG], v16d[h][:, 0:TPG])
                nc.sync.dma_start(v8[:, 0:TPG], v8d[h][:, 0:TPG])
                nc.sync.dma_start(qkt[:], qktd[h])
                nc.sync.dma_start(v16[:, TPG:], v16d[h][:, TPG:])
                nc.sync.dma_start(v8[:, TPG:], v8d[h][:, TPG:])

                def qk_cols(ti, c0, c1):
                    # columns [c0, c1) of packed tensor ti (0=q1,1=k1,2=q2,3=k2)
                    if c1 <= GW:
                        return qkf[:, ti, c0:c1]
                    assert c0 >= GW
                    return qkt[:, ti, c0 - GW : c1 - GW]

                for g in range(G):
                    jfull = TPG * g
                    qcols = [qk_cols(2 * pi, g * GW, (g + 1) * GW)
                             for pi in range(2)]
                    # both passes' accumulators in one 2-bank tile each, so
                    # the epilogue drains them with single wide DVE ops
                    outp_t = opool.tile([P, 2 * GW], fp32, tag="outp",
                                        name=f"outp_{h}_{g}")
                    sums_t = upool.tile([P, 2 * GW], fp32, tag="sums",
                                        name=f"sums_{h}_{g}")

                    # ---- full key-tile pairs: fp8 E + DoubleRow PV/sums ----
                    for pj in range(jfull // 2):
                        j0 = 2 * pj
                        for pi in range(2):
                            st = spool.tile([P, 1024], fp32, tag="st")
                            et = epool.tile([P, 1024], fp8, tag="et")
                            for dj in range(2):
                                nc.tensor.matmul(
                                    st[:, dj * GW : (dj + 1) * GW],
                                    qk_cols(2 * pi + 1, (j0 + dj) * P,
                                            (j0 + dj + 1) * P),
                                    qcols[pi],
                                    start=True,
                                    stop=True,
                                )
                            nc.scalar.activation(
                                et[:], st[:], Exp, scale=SCALE, bias=bias_s[:]
                            )
                            epair = et[:].rearrange("p (t q) -> p t q", t=2, q=GW)
                            nc.tensor.matmul(
                                sums_t[:, pi * GW : (pi + 1) * GW],
                                ones8[:], epair,
                                start=(pj == 0), stop=False,
                                perf_mode=DR, skip_group_check=True,
                            )
                            nc.tensor.matmul(
                                outp_t[:, pi * GW : (pi + 1) * GW],
                                v8[:, j0 : j0 + 2, :], epair,
                                start=(pj == 0), stop=False,
                                perf_mode=DR, skip_group_check=True,
                            )

                    # ---- diagonal: fp16, 3 allocations per group ----
                    # A/B (per pass): dr0 at [0:512], dr1 at [512:896]
                    # C (shared):   p0dr2 [0:256], p1dr2 [256:512],
                    #               p0dr3 [512:640], p1dr3 [640:768]
                    etds = []
                    for pi in range(2):
                        st = spool.tile([P, 1024], fp32, tag="st")
                        etd = edpool.tile([P, 896], fp16, tag=f"etd{pi}")
                        for dr, off in ((0, 0), (1, 512)):
                            j = jfull + dr
                            nc.tensor.matmul(
                                st[:, off : off + WIDS[dr]],
                                qk_cols(2 * pi + 1, j * P, (j + 1) * P),
                                qk_cols(2 * pi, g * GW + dr * P, (g + 1) * GW),
                                start=True, stop=False, skip_group_check=True,
                            )
                            band_kill(st, off)
                        nc.scalar.activation(
                            etd[:], st[:, 0:896], Exp,
                            scale=SCALE, bias=bias_s[:],
                        )
                        etds.append(etd)
                    stc = spool.tile([P, 1024], fp32, tag="st")
                    etc = edpool.tile([P, 768], fp16, tag="etdc")
                    for pi in range(2):
                        for dr, off in ((2, 256 * pi), (3, 512 + 128 * pi)):
                            j = jfull + dr
                            nc.tensor.matmul(
                                stc[:, off : off + WIDS[dr]],
                                qk_cols(2 * pi + 1, j * P, (j + 1) * P),
                                qk_cols(2 * pi, g * GW + dr * P, (g + 1) * GW),
                                start=True, stop=False, skip_group_check=True,
                            )
                            band_kill(stc, off)
                    nc.scalar.activation(
                        etc[:], stc[:, 0:768], Exp,
                        scale=SCALE, bias=bias_s[:],
                    )
                    for pi in range(2):
                        ecols = {
                            0: etds[pi][:, 0:512],
                            1: etds[pi][:, 512:896],
                            2: etc[:, 256 * pi : 256 * pi + 256],
                            3: etc[:, 512 + 128 * pi : 640 + 128 * pi],
                        }
                        for dr in range(TPG):
                            j = jfull + dr
                            nc.tensor.matmul(
                                sums_t[:, pi * GW + dr * P : (pi + 1) * GW],
                                ones16[:], ecols[dr],
                                start=(dr == 0 and jfull == 0),
                                stop=(dr == TPG - 1),
                                skip_group_check=True,
                            )
                            nc.tensor.matmul(
                                outp_t[:, pi * GW + dr * P : (pi + 1) * GW],
                                v16[:, j, :], ecols[dr],
                                start=(dr == 0 and jfull == 0),
                                stop=(dr == TPG - 1),
                                skip_group_check=True,
                            )

                    # ---- epilogue: fin = outp0/sums0 - lam*outp1/sums1 ----
                    # one wide reciprocal + one wide mul drain both passes'
                    # PSUM accumulators, minimizing how long the next group's
                    # first matmuls (start=True writers) stay blocked
                    rcp = fpool.tile([P, 2 * GW], fp32, tag="rcp")
                    nc.vector.reciprocal_approx_fast(rcp[:], sums_t[:])
                    t12 = fpool.tile([P, 2 * GW], fp32, tag="t12")
                    nc.vector.tensor_mul(t12[:], outp_t[:], rcp[:])
                    fin = fpool.tile([P, GW], fp32, tag="fin")
                    nc.vector.scalar_tensor_tensor(
                        fin[:], t12[:, GW:], neglam_s[:], t12[:, 0:GW],
                        op0=mybir.AluOpType.mult, op1=mybir.AluOpType.add,
                    )
                    nc.sync.dma_start(out[h][:, g * GW : (g + 1) * GW], fin[:])

    nc.compile()
    return nc


def _get_program():
    global _PROGRAM
    if _PROGRAM is None:
        _PROGRAM = _build_program()
    return _PROGRAM


def _make_in_maps(q1, k1, v, q2, k2, lambda_log):
    lam_val = float(np.exp(np.float64(lambda_log.reshape(-1)[0])))
    neglam_np = np.full((P, 1), -lam_val, dtype=np.float32)
    ident_np = np.eye(P, dtype=np.float16)
    # kill-mask for the diagonal band: -60000 where k > q (above causal diag)
    trib_np = np.where(
        np.arange(P)[:, None] > np.arange(P)[None, :], -60000.0, 0.0
    ).astype(np.float16)
    # combined band patterns for the shared dr2/dr3 diag allocation:
    # bank0 [0:384]: bands at 0 (p0dr2) and 256 (p1dr2);
    # bank1 [512:768]: bands at 512 (p0dr3) and 640 (p1dr3)
    bandc0_np = np.zeros((P, 384), dtype=np.float16)
    bandc0_np[:, 0:P] = trib_np
    bandc0_np[:, 256:384] = trib_np
    bandc1_np = np.zeros((P, 256), dtype=np.float16)
    bandc1_np[:, 0:P] = trib_np
    bandc1_np[:, P:256] = trib_np

    def t(x):  # [BH, S, D] -> [BH, D, S] contiguous fp16
        return np.ascontiguousarray(
            x.reshape(BH, S, D).transpose(0, 2, 1)
        ).astype(np.float16)

    q1t = t(q1)
    q2t = t(q2)
    k1t = t(k1)
    k2t = t(k2)
    qk4 = np.stack([q1t, k1t, q2t, k2t], axis=2)  # [BH, P, 4, S]
    qkf_np = np.ascontiguousarray(qk4[:, :, :, 0:GW])
    qkt_np = np.ascontiguousarray(qk4[:, :, :, GW:])
    # pre-tile V to [BH, p, j, d]: v_s[p, j, d] = V[128 j + p, d]
    vf = np.ascontiguousarray(v.reshape(BH, NT, P, D).transpose(0, 2, 1, 3))
    v16_np = vf.astype(np.float16)
    v8_np = vf.astype(ml_dtypes.float8_e4m3)

    in_maps = []
    for c in range(NCORES):
        sl = slice(c * HEADS, (c + 1) * HEADS)
        in_maps.append(
            {
                "qkf": qkf_np[sl],
                "qkt": qkt_np[sl],
                "v16": v16_np[sl],
                "v8": v8_np[sl],
                "neglam": neglam_np,
                "ident": ident_np,
                "trib": trib_np,
                "bandc0": bandc0_np,
                "bandc1": bandc1_np,
            }
        )
    return in_maps


def _run(q1, k1, v, q2, k2, lambda_log, trace=False):
    from concourse.bass_utils import run_bass_kernel_spmd

    nc = _get_program()
    in_maps = _make_in_maps(q1, k1, v, q2, k2, lambda_log)
    res = run_bass_kernel_spmd(
        nc, in_maps, core_ids=list(range(NCORES)), trace=trace
    )
    parts = [res.results[c]["out"].transpose(0, 2, 1) for c in range(NCORES)]
    full = np.concatenate(parts, axis=0).reshape(B, H, S, D)
    return np.ascontiguousarray(full, dtype=np.float32), res


def kernel(q1, k1, v, q2, k2, lambda_log):
    out, _ = _run(q1, k1, v, q2, k2, lambda_log, trace=False)
    return out
